# revision 38
# baseline (speedup 1.0000x reference)
"""Multi-head causal self-attention on 8 Trainium2 NeuronCores.

Sharding: heads 2-per-core (tensor parallel). Every core computes bf16 QKV
projections for its 2 heads over the full batch; attention runs per-batch
with both heads' score matmuls row-tiled into the PE array concurrently
(DK=64 stationaries at base partitions 0/64), fine-grained causal widths,
and an augmented ones-row on V so the softmax denominator falls out of the
same matmul. Outputs are redistributed with one 8-way AllToAll per batch
(the first overlaps the second batch's sweep), and the row-parallel output
projection runs per-batch so only the last small collective is exposed.

Reference semantics (torch nn.Linear convention, y = x @ W.T):
  Q = x @ Wq.T ; K = x @ Wk.T ; V = x @ Wv.T           (split into 16 heads)
  scores = Q K^T / sqrt(64), causal-masked, softmax
  out = (softmax(scores) @ V, concat heads) @ Wo.T + bo
"""

import sys
from collections import deque
from contextlib import ExitStack

sys.path.insert(0, "/opt/trn_rl_repo")

import numpy as np

import concourse.bass as bass_mod  # noqa: E402
import concourse.mybir as mybir  # noqa: E402
from concourse import bacc  # noqa: E402
from concourse.bass_utils import run_bass_kernel_spmd  # noqa: E402
from concourse.masks import make_identity  # noqa: E402
from concourse.tile import TileContext  # noqa: E402

B = 2
S = 2048
D = 1024
H = 16
DK = 64
N_CORES = 8
HPC = H // N_CORES          # heads per core = 2
EL = HPC * DK               # local embedding slice = 128
P = 128                     # partitions
SBLK = 512                  # q-block
NJ = S // SBLK              # q-blocks per batch = 4
NT = S // P                 # k-tiles per batch = 16
ND = D // P                 # d-chunks = 8
BS = B * S
QC = 256                    # output chunk columns (per-core rows per batch)
VW = DK + 1                 # V width incl ones-row = 65
F32 = mybir.dt.float32
BF16 = mybir.dt.bfloat16


def _build_program():
    nc = bacc.Bacc("TRN2", target_bir_lowering=False, debug=False,
                   num_devices=N_CORES)

    # ---- I/O (bf16 inputs pre-tiled on host) --------------------------
    xT = nc.declare_dram_parameter("xT", [D, BS], BF16, isOutput=False)
    wq = nc.declare_dram_parameter("wq", [P, ND * EL], BF16, isOutput=False)
    wk = nc.declare_dram_parameter("wk", [P, ND * EL], BF16, isOutput=False)
    wv = nc.declare_dram_parameter("wv", [P, ND * EL], BF16, isOutput=False)
    wo = nc.declare_dram_parameter("wo", [P, ND * D], BF16, isOutput=False)
    bo = nc.declare_dram_parameter("bo", [1, D], F32, isOutput=False)
    msk = nc.declare_dram_parameter("msk", [P, 2 * P], BF16, isOutput=False)
    out = nc.declare_dram_parameter("out", [2 * QC, D], F32, isOutput=True)

    a2a_in = [nc.dram_tensor(f"a2a_in{b}", [N_CORES, P, QC], BF16)
              for b in range(B)]
    a2a_out = [nc.dram_tensor(f"a2a_out{b}", [N_CORES, P, QC], BF16)
               for b in range(B)]

    with TileContext(nc) as tc, ExitStack() as ctx:
        const = ctx.enter_context(tc.tile_pool(name="const", bufs=1))
        persist = ctx.enter_context(tc.tile_pool(name="persist", bufs=1))
        xtp = ctx.enter_context(tc.tile_pool(name="xt", bufs=8))
        vtp = ctx.enter_context(tc.tile_pool(name="vt", bufs=2))
        probs_pool = ctx.enter_context(tc.tile_pool(name="probs", bufs=7))
        small = ctx.enter_context(tc.tile_pool(name="small", bufs=4))
        osb = ctx.enter_context(tc.tile_pool(name="osb", bufs=4))
        catp = ctx.enter_context(tc.tile_pool(name="cat", bufs=1))
        # PSUM: pp 2 + sc 2*2 + po0 1 + po1 1 = 8 banks
        pp = ctx.enter_context(tc.tile_pool(name="pp", bufs=2, space="PSUM"))
        scp = ctx.enter_context(tc.tile_pool(name="sc", bufs=2, space="PSUM"))
        pop = ctx.enter_context(tc.tile_pool(name="po", bufs=1, space="PSUM"))

        # ---- constants ------------------------------------------------
        identf = const.tile([P, P], F32, tag="identf")
        make_identity(nc, identf)
        ident = const.tile([P, P], BF16, tag="ident")
        nc.vector.tensor_copy(ident[:], identf[:])
        bo_sb = const.tile([1, D], F32, tag="bo_sb")
        bo_bc = const.tile([P, D], F32, tag="bo_bc")
        mask2 = const.tile([P, 2 * P], BF16, tag="mask2")
        w_sb = {}
        for name in ("wq", "wk", "wv"):
            w = const.tile([P, ND * EL], BF16, tag=f"w_{name}",
                           name=f"w_{name}")
            w_sb[name] = w
        wo_sb = const.tile([P, ND * D], BF16, tag="wo_sb")
        warm = const.tile([P, 640], BF16, tag="warm")
        nc.vector.memset(warm[:].bitcast(F32), 0.0)

        def load_consts_early():
            for i, (name, t) in enumerate((("wq", wq), ("wk", wk),
                                           ("wv", wv))):
                dma_engines[i % 3].dma_start(out=w_sb[name][:], in_=t[:])
            nc.scalar.dma_start(out=mask2[:], in_=msk[:])
            nc.sync.dma_start(out=bo_sb[:], in_=bo[:])
            nc.gpsimd.partition_broadcast(bo_bc[:], bo_sb[:])

        def load_consts_late():
            nc.sync.dma_start(out=wo_sb[:], in_=wo[:])

        # persistent activations
        qT = persist.tile([P, BS], BF16, tag="qT")
        kT = persist.tile([P, BS], BF16, tag="kT")
        # vaug[b]: 16 k-tiles of [P, 130]: cols 65h..65h+64 = V_h, 65h+64=1
        vaug = [persist.tile([P, NT * 2 * VW], BF16, tag=f"vaug{b}",
                             name=f"vaug{b}")
                for b in range(B)]
        stage = [persist.tile([P, SBLK], BF16, tag=f"stg{b}{j}",
                              name=f"stg{b}{j}")
                 for b in range(B) for j in range(NJ)]

        mask3 = mask2[:].rearrange("p (h q) -> p h q", h=2)

        # act table warm: tiny exp early so the ~2.7us table load overlaps
        # the initial x DMA
        warm_pr = small.tile([1, 32], F32, tag="wpr")
        nc.scalar.activation(warm_pr[:], warm[0:1, 0:32],
                             mybir.ActivationFunctionType.Exp)

        # ---- emission helpers -----------------------------------------
        warm_ctr = [0]

        def emit_warmup(n):
            for i in range(n):
                warm_ctr[0] += 1
                ps = pp.tile([P, SBLK], F32, tag="pp",
                             name=f"warm{warm_ctr[0]}")
                nc.tensor.matmul(ps[0:P, 0:SBLK], warm[:, 0:128],
                                 warm[:, 128:640], start=True, stop=True)

        dma_engines = [nc.sync, nc.scalar, nc.gpsimd]

        def load_x(b):
            xt = []
            for k in range(ND):
                t = xtp.tile([P, S], BF16, tag="xt")
                dma_engines[k % 3].dma_start(
                    out=t[:], in_=xT[k * P:(k + 1) * P, b * S:(b + 1) * S])
                xt.append(t)
            return xt

        def proj_groups(b, xt, vt, wide=False):
            """Yield once per (name, sb-group): stationary reused across
            2 (interleaved phase) or 4 (solo phase, borrowing sc banks)
            sub-blocks."""
            for sb0, name in ((a, n) for a in (0, 2)
                              for n in ("wq", "wk", "wv")):
                if wide:
                    sct = [scp.tile([P, 2 * SBLK], F32, tag="sc",
                                    name=f"pjw_{b}_{name}_{sb0}_{i}")
                           for i in range(2)]
                    pss = [sct[i][:, j * SBLK:(j + 1) * SBLK]
                           for i in range(2) for j in range(2)]
                    groups = [(sb0, pss)]
                else:
                    t2 = [pp.tile([P, SBLK], F32, tag="pp",
                                  name=f"pj_{b}_{name}_{sb0}_{i}")
                          for i in range(2)]
                    groups = [(sb0, [t[:] for t in t2])]
                for sb0, ps_group in groups:
                    nsb = len(ps_group)
                    for k in range(ND):
                        for i in range(nsb):
                            nc.tensor.matmul(
                                ps_group[i],
                                w_sb[name][:, k * EL:(k + 1) * EL],
                                xt[k][:, (sb0 + i) * SBLK:(sb0 + i + 1) * SBLK],
                                start=(k == 0), stop=(k == ND - 1))
                        if k == 3 and not wide:
                            yield
                    for i in range(nsb):
                        osl = slice(b * S + (sb0 + i) * SBLK,
                                    b * S + (sb0 + i + 1) * SBLK)
                        if name == "wv":
                            nc.vector.tensor_copy(
                                vt[:, (sb0 + i) * SBLK:(sb0 + i + 1) * SBLK],
                                ps_group[i])
                        else:
                            dest = qT if name == "wq" else kT
                            nc.vector.tensor_copy(dest[:, osl], ps_group[i])
                    yield

        def transpose_v(b, vt):
            """Yield once per pair of k-tiles transposed into vaug[b]."""
            va = vaug[b]
            # ones columns first (disjoint from the V columns written below)
            ones = va[:].rearrange("p (t h e) -> p t h e", t=NT, h=2)
            nc.vector.memset(ones[:, :, :, DK:DK + 1], 1.0)
            for t in range(NT):
                ps = pp.tile([P, SBLK], F32, tag="pp", name=f"tr_{b}_{t}")
                psb = ps[:].bitcast(BF16)
                nc.tensor.matmul(psb[:, 0:P], vt[:, t * P:(t + 1) * P],
                                 ident[:], is_transpose=True)
                va3 = va[:, t * 2 * VW:(t + 1) * 2 * VW].rearrange(
                    "p (h e) -> p h e", h=2)
                nc.vector.tensor_copy(
                    va3[:, :, 0:DK],
                    psb[:, 0:P].rearrange("p (h e) -> p h e", h=2))
                if t % 2 == 1:
                    yield

        pend = deque()
        LAG = 4
        po_live = {}

        def emit_attnv(b, j, t, pr3, off, w):
            if t == 0:
                po_live[(b, j)] = {
                    h: pop.tile([P, SBLK], F32, tag=f"po{h}",
                                name=f"po_{b}_{j}_{h}")
                    for h in (0, 1)}
            po = po_live[(b, j)]
            last = (t == 4 * j + 3)
            for h in (0, 1):
                nc.tensor.matmul(
                    po[h][0:VW, off:SBLK],
                    vaug[b][:, (t * 2 + h) * VW:(t * 2 + h) * VW + VW],
                    pr3[:, h, off:SBLK],
                    start=(t == 0), stop=last)
            if last:
                emit_norm(b, j, po)
                del po_live[(b, j)]

        def emit_norm(b, j, po):
            import os
            st = stage[b * NJ + j]
            for h in (0, 1):
                if os.environ.get("KDBG", "") == "raw":
                    nc.vector.tensor_copy(st[h * DK:(h + 1) * DK, :],
                                          po[h][0:DK, :])
                    continue
                if os.environ.get("KDBG", "") == "den":
                    nc.vector.tensor_copy(
                        st[h * DK:h * DK + 1, :], po[h][DK:DK + 1, :])
                    continue
                sm = small.tile([1, SBLK], F32, tag="sm")
                nc.vector.tensor_copy(sm[:], po[h][DK:DK + 1, :])
                rb1 = small.tile([1, SBLK], F32, tag="rb1")
                nc.vector.reciprocal_approx_fast(out=rb1[:], in_=sm[:])
                rbb = small.tile([DK, SBLK], F32, tag="rbb")
                nc.gpsimd.partition_broadcast(rbb[:], rb1[:])
                nc.vector.tensor_mul(st[h * DK:(h + 1) * DK, :],
                                     po[h][0:DK, :], rbb[:])
            for half in (0, 1):
                nc.sync.dma_start(
                    out=a2a_in[b][2 * j + half],
                    in_=st[:, half * QC:(half + 1) * QC])

        def sweep_units(b):
            """Yield once per (j, t) unit: score pair + exp (+mask); attnV
            lags via pend."""
            for j in range(NJ):
                for t in range(4 * j + 4):
                    r = t - 4 * j
                    off = max(0, r) * P
                    w = SBLK - off
                    sc = scp.tile([P, 2 * SBLK], F32, tag="sc",
                                  name=f"sc_{b}_{j}_{t}")
                    sc3 = sc[:].rearrange("p (h q) -> p h q", h=2)
                    q0 = b * S + j * SBLK + off
                    k0 = b * S + t * P
                    for h in (0, 1):
                        nc.tensor.matmul(
                            sc3[:, h, off:SBLK],
                            kT[h * DK:(h + 1) * DK, k0:k0 + P],
                            qT[h * DK:(h + 1) * DK, q0:q0 + w],
                            start=True, stop=True)
                    pr = probs_pool.tile([P, 2 * SBLK], BF16, tag="probs")
                    pr3 = pr[:].rearrange("p (h q) -> p h q", h=2)
                    nc.scalar.activation(pr3[:, :, off:SBLK],
                                         sc3[:, :, off:SBLK],
                                         mybir.ActivationFunctionType.Exp)
                    if r >= 0:
                        nc.vector.tensor_mul(pr3[:, :, off:off + P],
                                             pr3[:, :, off:off + P],
                                             mask3)
                    pend.append((b, j, t, pr3, off, w))
                    # previous q-block's attnVs are past their exps: drain
                    # them now so the new block's first attnV (which waits
                    # on the normalize chain freeing the po banks) sits
                    # deep enough in the FIFO to hide that chain
                    while pend and pend[0][1] != j:
                        emit_attnv(*pend.popleft())
                    while len(pend) > LAG:
                        emit_attnv(*pend.popleft())
                    yield

        def drain():
            while pend:
                emit_attnv(*pend.popleft())

        def load_cat(b, engines):
            cat = []
            for s in range(N_CORES):
                t = catp.tile([P, QC], BF16, tag=f"cat{b}{s}",
                              name=f"cat{b}{s}")
                engines[s % len(engines)].dma_start(out=t[:],
                                                    in_=a2a_out[b][s])
                cat.append(t)
            return cat

        def outproj(b, cat):
            """Yield per (st, eb) block of the output projection for batch
            b's 2x128 rows."""
            for st in (0, 1):
                pss = [pp.tile([P, SBLK], F32, tag="pp",
                               name=f"op_{b}_{st}_{eb}") for eb in (0, 1)]
                for s in range(N_CORES):
                    for eb in (0, 1):
                        nc.tensor.matmul(
                            pss[eb][:], cat[s][:, st * P:(st + 1) * P],
                            wo_sb[:, s * D + eb * SBLK:s * D + (eb + 1) * SBLK],
                            start=(s == 0), stop=(s == N_CORES - 1))
                for eb in (0, 1):
                    ot = osb.tile([P, SBLK], F32, tag="osb")
                    nc.vector.tensor_add(ot[:], pss[eb][:],
                                         bo_bc[:, eb * SBLK:(eb + 1) * SBLK])
                    nc.sync.dma_start(
                        out=out[b * QC + st * P:b * QC + (st + 1) * P,
                                eb * SBLK:(eb + 1) * SBLK],
                        in_=ot[:])
                    yield

        # ---- master schedule ------------------------------------------
        xt0 = load_x(0)
        load_consts_early()
        emit_warmup(26)
        vt0 = vtp.tile([P, S], BF16, tag="vt", name="vt0")
        for _ in proj_groups(0, xt0, vt0):
            pass
        for _ in transpose_v(0, vt0):
            pass

        xt1 = load_x(1)
        load_consts_late()
        vt1 = vtp.tile([P, S], BF16, tag="vt", name="vt1")
        import os
        KSEQ = os.environ.get("KSEQ", "0")
        SEQ = KSEQ in ("1", "3")      # sequential proj-b1/tv1
        SEQ_OP = KSEQ in ("1", "2")   # sequential outproj-b0
        sw0 = sweep_units(0)
        pj1 = proj_groups(1, xt1, vt1)
        tv1 = transpose_v(1, vt1)
        if SEQ:
            for _ in pj1:
                pass
            for _ in tv1:
                pass
            for _ in sw0:
                pass
        else:
            for i in range(40):
                next(sw0, None)
                if i % 2 == 0 and i < 24:
                    next(pj1, None)
                elif i % 2 == 0:
                    next(tv1, None)
            for _ in sw0:
                pass
            for _ in pj1:
                pass
            for _ in tv1:
                pass
        drain()
        nc.gpsimd.collective_compute(
            "AllToAll", mybir.AluOpType.bypass,
            replica_groups=[list(range(N_CORES))],
            ins=[a2a_in[0][:]], outs=[a2a_out[0][:]])
        cat0 = load_cat(0, [nc.sync])

        # sweep b1 interleaved with the deferred sb23 projections and
        # V-transposes (fills the Act-bound phase's PE bubbles); then a2a#B
        # launches while outproj b0 (whose collective landed mid-sweep)
        # keeps the PE busy
        for _ in sweep_units(1):
            pass
        drain()
        nc.gpsimd.collective_compute(
            "AllToAll", mybir.AluOpType.bypass,
            replica_groups=[list(range(N_CORES))],
            ins=[a2a_in[1][:]], outs=[a2a_out[1][:]])
        cat1 = load_cat(1, [nc.sync, nc.scalar])
        tc.no_sync_barrier()
        for _ in outproj(0, cat0):
            pass
        for _ in outproj(1, cat1):
            pass

        import os
        dbg = os.environ.get("KDBG", "")
        if dbg in ("q", "k", "v", "st", "raw", "den"):
            dbt = osb.tile([P, D], F32, tag="dbg", name="dbt")
            if dbg == "q":
                srcs = [qT[:, 0:D], qT[:, S:S + D]]
            elif dbg == "k":
                srcs = [kT[:, 0:D], kT[:, S:S + D]]
            elif dbg == "v":
                srcs = [vaug[0][:, 0:D], vaug[1][:, 0:D]]
            elif dbg in ("st", "raw", "den"):
                srcs = [stage[0][:], stage[3][:], stage[4][:], stage[7][:]]
            for r, sap in enumerate(srcs):
                w = sap.free_size()
                nc.vector.tensor_copy(dbt[:, 0:w], sap)
                nc.sync.dma_start(out=out[r * P:(r + 1) * P, 0:w],
                                  in_=dbt[:, 0:w])

    nc.compile()
    return nc


def _sbuf_tiled(wT):
    # [D, E] -> [P, ND*E]: row p holds d-chunk k at columns [k*E, (k+1)*E)
    dd, e = wT.shape
    return np.ascontiguousarray(
        wT.reshape(dd // P, P, e).transpose(1, 0, 2).reshape(P, -1))


def _bf16(a):
    import ml_dtypes
    return np.ascontiguousarray(a).astype(ml_dtypes.bfloat16)


def _prepare_inputs(x, Wq, Wk, Wv, Wo, bo):
    x = np.asarray(x, np.float32)
    xT = np.ascontiguousarray(
        np.concatenate([x[b].T for b in range(B)], axis=1))
    xT = _bf16(xT)
    woT = _bf16(_sbuf_tiled(np.ascontiguousarray(np.asarray(Wo,
                                                            np.float32).T)))
    bo2 = np.asarray(bo, np.float32).reshape(1, D)
    scale = np.float32(1.0 / np.sqrt(DK))
    # causal triangle in [k, q] layout: valid iff q >= k; two copies side
    # by side so one DVE multiply masks both heads
    tri = np.triu(np.ones((P, P), np.float32))
    msk = _bf16(np.concatenate([tri, tri], axis=1))
    in_maps = []
    for c in range(N_CORES):
        cols = slice(c * EL, (c + 1) * EL)
        in_maps.append({
            "xT": xT,
            "wq": _bf16(_sbuf_tiled(np.asarray(Wq, np.float32).T[:, cols]
                                    * scale)),
            "wk": _bf16(_sbuf_tiled(np.asarray(Wk, np.float32).T[:, cols])),
            "wv": _bf16(_sbuf_tiled(np.asarray(Wv, np.float32).T[:, cols])),
            "wo": woT,
            "bo": bo2,
            "msk": msk,
        })
    return in_maps


def _run(inputs, trace=False):
    nc = _build_program()
    in_maps = _prepare_inputs(inputs["x"], inputs["Wq"], inputs["Wk"],
                              inputs["Wv"], inputs["Wo"], inputs["bo"])
    res = run_bass_kernel_spmd(nc, in_maps, list(range(N_CORES)),
                               trace=trace)
    full = np.empty((B, S, D), np.float32)
    for c in range(N_CORES):
        r = res.results[c]["out"]
        for b in range(B):
            full[b, c * QC:(c + 1) * QC, :] = r[b * QC:(b + 1) * QC]
    return full, res


def kernel(**inputs) -> np.ndarray:
    out, _ = _run(inputs, trace=False)
    return out


# revision 39
# speedup vs baseline: 1.0277x; 1.0277x over previous
"""Multi-head causal self-attention on 8 Trainium2 NeuronCores.

Sharding: heads 2-per-core (tensor parallel). Every core computes bf16 QKV
projections for its 2 heads over the full batch; attention runs per-batch
with both heads' score matmuls row-tiled into the PE array concurrently
(DK=64 stationaries at base partitions 0/64), fine-grained causal widths,
and an augmented ones-row on V so the softmax denominator falls out of the
same matmul. Outputs are redistributed with one 8-way AllToAll per batch
(the first overlaps the second batch's sweep), and the row-parallel output
projection runs per-batch so only the last small collective is exposed.

Reference semantics (torch nn.Linear convention, y = x @ W.T):
  Q = x @ Wq.T ; K = x @ Wk.T ; V = x @ Wv.T           (split into 16 heads)
  scores = Q K^T / sqrt(64), causal-masked, softmax
  out = (softmax(scores) @ V, concat heads) @ Wo.T + bo
"""

import sys
from collections import deque
from contextlib import ExitStack

sys.path.insert(0, "/opt/trn_rl_repo")

import numpy as np

import concourse.bass as bass_mod  # noqa: E402
import concourse.mybir as mybir  # noqa: E402
from concourse import bacc  # noqa: E402
from concourse.bass_utils import run_bass_kernel_spmd  # noqa: E402
from concourse.masks import make_identity  # noqa: E402
from concourse.tile import TileContext  # noqa: E402

B = 2
S = 2048
D = 1024
H = 16
DK = 64
N_CORES = 8
HPC = H // N_CORES          # heads per core = 2
EL = HPC * DK               # local embedding slice = 128
P = 128                     # partitions
SBLK = 512                  # q-block
NJ = S // SBLK              # q-blocks per batch = 4
NT = S // P                 # k-tiles per batch = 16
ND = D // P                 # d-chunks = 8
BS = B * S
QC = 256                    # output chunk columns (per-core rows per batch)
VW = DK + 1                 # V width incl ones-row = 65
F32 = mybir.dt.float32
BF16 = mybir.dt.bfloat16


def _build_program():
    nc = bacc.Bacc("TRN2", target_bir_lowering=False, debug=False,
                   num_devices=N_CORES)

    # ---- I/O (bf16 inputs pre-tiled on host) --------------------------
    xT = nc.declare_dram_parameter("xT", [D, BS], BF16, isOutput=False)
    wq = nc.declare_dram_parameter("wq", [P, ND * EL], BF16, isOutput=False)
    wk = nc.declare_dram_parameter("wk", [P, ND * EL], BF16, isOutput=False)
    wv = nc.declare_dram_parameter("wv", [P, ND * EL], BF16, isOutput=False)
    wo = nc.declare_dram_parameter("wo", [P, ND * D], BF16, isOutput=False)
    bo = nc.declare_dram_parameter("bo", [1, D], F32, isOutput=False)
    msk = nc.declare_dram_parameter("msk", [P, 2 * P], BF16, isOutput=False)
    out = nc.declare_dram_parameter("out", [2 * QC, D], F32, isOutput=True)

    a2a_in = [nc.dram_tensor(f"a2a_in{b}", [N_CORES, P, QC], BF16)
              for b in range(B)]
    a2a_out = [nc.dram_tensor(f"a2a_out{b}", [N_CORES, P, QC], BF16)
               for b in range(B)]

    with TileContext(nc) as tc, ExitStack() as ctx:
        const = ctx.enter_context(tc.tile_pool(name="const", bufs=1))
        persist = ctx.enter_context(tc.tile_pool(name="persist", bufs=1))
        xtp = ctx.enter_context(tc.tile_pool(name="xt", bufs=8))
        vtp = ctx.enter_context(tc.tile_pool(name="vt", bufs=2))
        probs_pool = ctx.enter_context(tc.tile_pool(name="probs", bufs=7))
        small = ctx.enter_context(tc.tile_pool(name="small", bufs=4))
        osb = ctx.enter_context(tc.tile_pool(name="osb", bufs=4))
        catp = ctx.enter_context(tc.tile_pool(name="cat", bufs=1))
        # PSUM: pp 2 + sc 2*2 + po0 1 + po1 1 = 8 banks
        pp = ctx.enter_context(tc.tile_pool(name="pp", bufs=2, space="PSUM"))
        scp = ctx.enter_context(tc.tile_pool(name="sc", bufs=2, space="PSUM"))
        pop = ctx.enter_context(tc.tile_pool(name="po", bufs=1, space="PSUM"))

        # ---- constants ------------------------------------------------
        identf = const.tile([P, P], F32, tag="identf")
        make_identity(nc, identf)
        ident = const.tile([P, P], BF16, tag="ident")
        nc.vector.tensor_copy(ident[:], identf[:])
        bo_sb = const.tile([1, D], F32, tag="bo_sb")
        bo_bc = const.tile([P, D], F32, tag="bo_bc")
        mask2 = const.tile([P, 2 * P], BF16, tag="mask2")
        w_sb = {}
        for name in ("wq", "wk", "wv"):
            w = const.tile([P, ND * EL], BF16, tag=f"w_{name}",
                           name=f"w_{name}")
            w_sb[name] = w
        wo_sb = const.tile([P, ND * D], BF16, tag="wo_sb")
        warm = const.tile([P, 640], BF16, tag="warm")
        nc.vector.memset(warm[:].bitcast(F32), 0.0)

        def load_consts_early():
            for i, (name, t) in enumerate((("wq", wq), ("wk", wk),
                                           ("wv", wv))):
                dma_engines[i % 3].dma_start(out=w_sb[name][:], in_=t[:])
            nc.scalar.dma_start(out=mask2[:], in_=msk[:])
            nc.sync.dma_start(out=bo_sb[:], in_=bo[:])
            nc.gpsimd.partition_broadcast(bo_bc[:], bo_sb[:])

        def load_consts_late():
            nc.sync.dma_start(out=wo_sb[:], in_=wo[:])

        # persistent activations
        qT = persist.tile([P, BS], BF16, tag="qT")
        kT = persist.tile([P, BS], BF16, tag="kT")
        # vaug[b]: 16 k-tiles of [P, 130]: cols 65h..65h+64 = V_h, 65h+64=1
        vaug = [persist.tile([P, NT * 2 * VW], BF16, tag=f"vaug{b}",
                             name=f"vaug{b}")
                for b in range(B)]
        stage = [persist.tile([P, SBLK], BF16, tag=f"stg{b}{j}",
                              name=f"stg{b}{j}")
                 for b in range(B) for j in range(NJ)]

        mask3 = mask2[:].rearrange("p (h q) -> p h q", h=2)

        # act table warm: tiny exp early so the ~2.7us table load overlaps
        # the initial x DMA
        warm_pr = small.tile([1, 32], F32, tag="wpr")
        nc.scalar.activation(warm_pr[:], warm[0:1, 0:32],
                             mybir.ActivationFunctionType.Exp)

        # ---- emission helpers -----------------------------------------
        warm_ctr = [0]

        def emit_warmup(n):
            for i in range(n):
                warm_ctr[0] += 1
                ps = pp.tile([P, SBLK], F32, tag="pp",
                             name=f"warm{warm_ctr[0]}")
                nc.tensor.matmul(ps[0:P, 0:SBLK], warm[:, 0:128],
                                 warm[:, 128:640], start=True, stop=True)

        dma_engines = [nc.sync, nc.scalar, nc.gpsimd]

        def load_x(b):
            xt = []
            for k in range(ND):
                t = xtp.tile([P, S], BF16, tag="xt")
                dma_engines[k % 3].dma_start(
                    out=t[:], in_=xT[k * P:(k + 1) * P, b * S:(b + 1) * S])
                xt.append(t)
            return xt

        def proj_groups(b, xt, vt, wide=False):
            """Yield once per (name, sb-group): stationary reused across
            2 (interleaved phase) or 4 (solo phase, borrowing sc banks)
            sub-blocks."""
            for sb0, name in ((a, n) for a in (0, 2)
                              for n in ("wq", "wk", "wv")):
                if wide:
                    sct = [scp.tile([P, 2 * SBLK], F32, tag="sc",
                                    name=f"pjw_{b}_{name}_{sb0}_{i}")
                           for i in range(2)]
                    pss = [sct[i][:, j * SBLK:(j + 1) * SBLK]
                           for i in range(2) for j in range(2)]
                    groups = [(sb0, pss)]
                else:
                    t2 = [pp.tile([P, SBLK], F32, tag="pp",
                                  name=f"pj_{b}_{name}_{sb0}_{i}")
                          for i in range(2)]
                    groups = [(sb0, [t[:] for t in t2])]
                for sb0, ps_group in groups:
                    nsb = len(ps_group)
                    for k in range(ND):
                        for i in range(nsb):
                            nc.tensor.matmul(
                                ps_group[i],
                                w_sb[name][:, k * EL:(k + 1) * EL],
                                xt[k][:, (sb0 + i) * SBLK:(sb0 + i + 1) * SBLK],
                                start=(k == 0), stop=(k == ND - 1))
                        if k == 3 and not wide:
                            yield
                    for i in range(nsb):
                        osl = slice(b * S + (sb0 + i) * SBLK,
                                    b * S + (sb0 + i + 1) * SBLK)
                        if name == "wv":
                            nc.vector.tensor_copy(
                                vt[:, (sb0 + i) * SBLK:(sb0 + i + 1) * SBLK],
                                ps_group[i])
                        else:
                            dest = qT if name == "wq" else kT
                            nc.vector.tensor_copy(dest[:, osl], ps_group[i])
                    yield

        def transpose_v(b, vt):
            """Yield once per pair of k-tiles transposed into vaug[b]."""
            va = vaug[b]
            # ones columns first (disjoint from the V columns written below)
            ones = va[:].rearrange("p (t h e) -> p t h e", t=NT, h=2)
            nc.vector.memset(ones[:, :, :, DK:DK + 1], 1.0)
            for t in range(NT):
                ps = pp.tile([P, SBLK], F32, tag="pp", name=f"tr_{b}_{t}")
                psb = ps[:].bitcast(BF16)
                nc.tensor.matmul(psb[:, 0:P], vt[:, t * P:(t + 1) * P],
                                 ident[:], is_transpose=True)
                va3 = va[:, t * 2 * VW:(t + 1) * 2 * VW].rearrange(
                    "p (h e) -> p h e", h=2)
                nc.vector.tensor_copy(
                    va3[:, :, 0:DK],
                    psb[:, 0:P].rearrange("p (h e) -> p h e", h=2))
                if t % 2 == 1:
                    yield

        pend = deque()
        LAG = 4
        po_live = {}

        def emit_attnv(b, j, t, pr3, off, w):
            if t == 0:
                po_live[(b, j)] = {
                    h: pop.tile([P, SBLK], F32, tag=f"po{h}",
                                name=f"po_{b}_{j}_{h}")
                    for h in (0, 1)}
            po = po_live[(b, j)]
            last = (t == 4 * j + 3)
            for h in (0, 1):
                nc.tensor.matmul(
                    po[h][0:VW, off:SBLK],
                    vaug[b][:, (t * 2 + h) * VW:(t * 2 + h) * VW + VW],
                    pr3[:, h, off:SBLK],
                    start=(t == 0), stop=last)
            if last:
                emit_norm(b, j, po)
                del po_live[(b, j)]

        def emit_norm(b, j, po):
            import os
            st = stage[b * NJ + j]
            for h in (0, 1):
                if os.environ.get("KDBG", "") == "raw":
                    nc.vector.tensor_copy(st[h * DK:(h + 1) * DK, :],
                                          po[h][0:DK, :])
                    continue
                if os.environ.get("KDBG", "") == "den":
                    nc.vector.tensor_copy(
                        st[h * DK:h * DK + 1, :], po[h][DK:DK + 1, :])
                    continue
                sm = small.tile([1, SBLK], F32, tag="sm")
                nc.vector.tensor_copy(sm[:], po[h][DK:DK + 1, :])
                rb1 = small.tile([1, SBLK], F32, tag="rb1")
                nc.vector.reciprocal_approx_fast(out=rb1[:], in_=sm[:])
                rbb = small.tile([DK, SBLK], F32, tag="rbb")
                nc.gpsimd.partition_broadcast(rbb[:], rb1[:])
                nc.vector.tensor_mul(st[h * DK:(h + 1) * DK, :],
                                     po[h][0:DK, :], rbb[:])
            for half in (0, 1):
                nc.sync.dma_start(
                    out=a2a_in[b][2 * j + half],
                    in_=st[:, half * QC:(half + 1) * QC])

        def sweep_units(b):
            """Yield once per (j, t) unit: score pair + exp (+mask); attnV
            lags via pend."""
            for j in range(NJ):
                for t in range(4 * j + 4):
                    r = t - 4 * j
                    off = max(0, r) * P
                    w = SBLK - off
                    sc = scp.tile([P, 2 * SBLK], F32, tag="sc",
                                  name=f"sc_{b}_{j}_{t}")
                    sc3 = sc[:].rearrange("p (h q) -> p h q", h=2)
                    q0 = b * S + j * SBLK + off
                    k0 = b * S + t * P
                    for h in (0, 1):
                        nc.tensor.matmul(
                            sc3[:, h, off:SBLK],
                            kT[h * DK:(h + 1) * DK, k0:k0 + P],
                            qT[h * DK:(h + 1) * DK, q0:q0 + w],
                            start=True, stop=True)
                    pr = probs_pool.tile([P, 2 * SBLK], BF16, tag="probs")
                    pr3 = pr[:].rearrange("p (h q) -> p h q", h=2)
                    nc.scalar.activation(pr3[:, :, off:SBLK],
                                         sc3[:, :, off:SBLK],
                                         mybir.ActivationFunctionType.Exp)
                    if r >= 0:
                        nc.vector.tensor_mul(pr3[:, :, off:off + P],
                                             pr3[:, :, off:off + P],
                                             mask3)
                    pend.append((b, j, t, pr3, off, w))
                    # previous q-block's attnVs are past their exps: drain
                    # them now so the new block's first attnV (which waits
                    # on the normalize chain freeing the po banks) sits
                    # deep enough in the FIFO to hide that chain
                    while pend and pend[0][1] != j:
                        emit_attnv(*pend.popleft())
                    while len(pend) > LAG:
                        emit_attnv(*pend.popleft())
                    yield

        def drain():
            while pend:
                emit_attnv(*pend.popleft())

        def load_cat(b, engines):
            cat = []
            for s in range(N_CORES):
                t = catp.tile([P, QC], BF16, tag=f"cat{b}{s}",
                              name=f"cat{b}{s}")
                engines[s % len(engines)].dma_start(out=t[:],
                                                    in_=a2a_out[b][s])
                cat.append(t)
            return cat

        def outproj(b, cat):
            """Yield per (st, eb) block of the output projection for batch
            b's 2x128 rows."""
            for st in (0, 1):
                pss = [pp.tile([P, SBLK], F32, tag="pp",
                               name=f"op_{b}_{st}_{eb}") for eb in (0, 1)]
                for s in range(N_CORES):
                    for eb in (0, 1):
                        nc.tensor.matmul(
                            pss[eb][:], cat[s][:, st * P:(st + 1) * P],
                            wo_sb[:, s * D + eb * SBLK:s * D + (eb + 1) * SBLK],
                            start=(s == 0), stop=(s == N_CORES - 1))
                for eb in (0, 1):
                    ot = osb.tile([P, SBLK], F32, tag="osb")
                    nc.vector.tensor_add(ot[:], pss[eb][:],
                                         bo_bc[:, eb * SBLK:(eb + 1) * SBLK])
                    nc.sync.dma_start(
                        out=out[b * QC + st * P:b * QC + (st + 1) * P,
                                eb * SBLK:(eb + 1) * SBLK],
                        in_=ot[:])
                    yield

        # ---- master schedule ------------------------------------------
        xt0 = load_x(0)
        load_consts_early()
        emit_warmup(26)
        vt0 = vtp.tile([P, S], BF16, tag="vt", name="vt0")
        pj0 = proj_groups(0, xt0, vt0)
        tv0 = transpose_v(0, vt0)
        sw0 = sweep_units(0)
        # sb0/sb1 projections + their transposes first, so sweep-b0's first
        # two q-blocks can run inside the projection phase
        for _ in range(6):
            next(pj0, None)
        for _ in range(4):
            next(tv0, None)

        xt1 = load_x(1)
        load_consts_late()
        vt1 = vtp.tile([P, S], BF16, tag="vt", name="vt1")
        import os
        KSEQ = os.environ.get("KSEQ", "0")
        SEQ = KSEQ in ("1", "3")      # sequential proj-b1/tv1
        SEQ_OP = KSEQ in ("1", "2")   # sequential outproj-b0
        pj1 = proj_groups(1, xt1, vt1)
        tv1 = transpose_v(1, vt1)
        # units 0-11 (q-blocks 0-1) interleave with the sb2/sb3 projections
        items0 = [pj0] * 6 + [tv0] * 4
        for i in range(12):
            next(sw0, None)
            if i < len(items0):
                next(items0[i], None)
        for _ in pj0:
            pass
        for _ in tv0:
            pass
        if SEQ:
            for _ in pj1:
                pass
            for _ in tv1:
                pass
            for _ in sw0:
                pass
        else:
            items1 = [pj1] * 6 + [tv1] * 4 + [pj1] * 6 + [tv1] * 4
            for i in range(28):
                next(sw0, None)
                if i < len(items1):
                    next(items1[i], None)
            for _ in sw0:
                pass
            for _ in pj1:
                pass
            for _ in tv1:
                pass
        drain()
        nc.gpsimd.collective_compute(
            "AllToAll", mybir.AluOpType.bypass,
            replica_groups=[list(range(N_CORES))],
            ins=[a2a_in[0][:]], outs=[a2a_out[0][:]])
        cat0 = load_cat(0, [nc.sync])

        # sweep b1 interleaved with the deferred sb23 projections and
        # V-transposes (fills the Act-bound phase's PE bubbles); then a2a#B
        # launches while outproj b0 (whose collective landed mid-sweep)
        # keeps the PE busy
        for _ in sweep_units(1):
            pass
        drain()
        nc.gpsimd.collective_compute(
            "AllToAll", mybir.AluOpType.bypass,
            replica_groups=[list(range(N_CORES))],
            ins=[a2a_in[1][:]], outs=[a2a_out[1][:]])
        cat1 = load_cat(1, [nc.sync, nc.scalar])
        tc.no_sync_barrier()
        for _ in outproj(0, cat0):
            pass
        for _ in outproj(1, cat1):
            pass

        import os
        dbg = os.environ.get("KDBG", "")
        if dbg in ("q", "k", "v", "st", "raw", "den"):
            dbt = osb.tile([P, D], F32, tag="dbg", name="dbt")
            if dbg == "q":
                srcs = [qT[:, 0:D], qT[:, S:S + D]]
            elif dbg == "k":
                srcs = [kT[:, 0:D], kT[:, S:S + D]]
            elif dbg == "v":
                srcs = [vaug[0][:, 0:D], vaug[1][:, 0:D]]
            elif dbg in ("st", "raw", "den"):
                srcs = [stage[0][:], stage[3][:], stage[4][:], stage[7][:]]
            for r, sap in enumerate(srcs):
                w = sap.free_size()
                nc.vector.tensor_copy(dbt[:, 0:w], sap)
                nc.sync.dma_start(out=out[r * P:(r + 1) * P, 0:w],
                                  in_=dbt[:, 0:w])

    nc.compile()
    return nc


def _sbuf_tiled(wT):
    # [D, E] -> [P, ND*E]: row p holds d-chunk k at columns [k*E, (k+1)*E)
    dd, e = wT.shape
    return np.ascontiguousarray(
        wT.reshape(dd // P, P, e).transpose(1, 0, 2).reshape(P, -1))


def _bf16(a):
    import ml_dtypes
    return np.ascontiguousarray(a).astype(ml_dtypes.bfloat16)


def _prepare_inputs(x, Wq, Wk, Wv, Wo, bo):
    x = np.asarray(x, np.float32)
    xT = np.ascontiguousarray(
        np.concatenate([x[b].T for b in range(B)], axis=1))
    xT = _bf16(xT)
    woT = _bf16(_sbuf_tiled(np.ascontiguousarray(np.asarray(Wo,
                                                            np.float32).T)))
    bo2 = np.asarray(bo, np.float32).reshape(1, D)
    scale = np.float32(1.0 / np.sqrt(DK))
    # causal triangle in [k, q] layout: valid iff q >= k; two copies side
    # by side so one DVE multiply masks both heads
    tri = np.triu(np.ones((P, P), np.float32))
    msk = _bf16(np.concatenate([tri, tri], axis=1))
    in_maps = []
    for c in range(N_CORES):
        cols = slice(c * EL, (c + 1) * EL)
        in_maps.append({
            "xT": xT,
            "wq": _bf16(_sbuf_tiled(np.asarray(Wq, np.float32).T[:, cols]
                                    * scale)),
            "wk": _bf16(_sbuf_tiled(np.asarray(Wk, np.float32).T[:, cols])),
            "wv": _bf16(_sbuf_tiled(np.asarray(Wv, np.float32).T[:, cols])),
            "wo": woT,
            "bo": bo2,
            "msk": msk,
        })
    return in_maps


def _run(inputs, trace=False):
    nc = _build_program()
    in_maps = _prepare_inputs(inputs["x"], inputs["Wq"], inputs["Wk"],
                              inputs["Wv"], inputs["Wo"], inputs["bo"])
    res = run_bass_kernel_spmd(nc, in_maps, list(range(N_CORES)),
                               trace=trace)
    full = np.empty((B, S, D), np.float32)
    for c in range(N_CORES):
        r = res.results[c]["out"]
        for b in range(B):
            full[b, c * QC:(c + 1) * QC, :] = r[b * QC:(b + 1) * QC]
    return full, res


def kernel(**inputs) -> np.ndarray:
    out, _ = _run(inputs, trace=False)
    return out


# revision 40
# speedup vs baseline: 1.0310x; 1.0032x over previous
"""Multi-head causal self-attention on 8 Trainium2 NeuronCores.

Sharding: heads 2-per-core (tensor parallel). Every core computes bf16 QKV
projections for its 2 heads over the full batch; attention runs per-batch
with both heads' score matmuls row-tiled into the PE array concurrently
(DK=64 stationaries at base partitions 0/64), fine-grained causal widths,
and an augmented ones-row on V so the softmax denominator falls out of the
same matmul. Outputs are redistributed with one 8-way AllToAll per batch
(the first overlaps the second batch's sweep), and the row-parallel output
projection runs per-batch so only the last small collective is exposed.

Reference semantics (torch nn.Linear convention, y = x @ W.T):
  Q = x @ Wq.T ; K = x @ Wk.T ; V = x @ Wv.T           (split into 16 heads)
  scores = Q K^T / sqrt(64), causal-masked, softmax
  out = (softmax(scores) @ V, concat heads) @ Wo.T + bo
"""

import sys
from collections import deque
from contextlib import ExitStack

sys.path.insert(0, "/opt/trn_rl_repo")

import numpy as np

import concourse.bass as bass_mod  # noqa: E402
import concourse.mybir as mybir  # noqa: E402
from concourse import bacc  # noqa: E402
from concourse.bass_utils import run_bass_kernel_spmd  # noqa: E402
from concourse.masks import make_identity  # noqa: E402
from concourse.tile import TileContext  # noqa: E402

B = 2
S = 2048
D = 1024
H = 16
DK = 64
N_CORES = 8
HPC = H // N_CORES          # heads per core = 2
EL = HPC * DK               # local embedding slice = 128
P = 128                     # partitions
SBLK = 512                  # q-block
NJ = S // SBLK              # q-blocks per batch = 4
NT = S // P                 # k-tiles per batch = 16
ND = D // P                 # d-chunks = 8
BS = B * S
QC = 256                    # output chunk columns (per-core rows per batch)
VW = DK + 1                 # V width incl ones-row = 65
F32 = mybir.dt.float32
BF16 = mybir.dt.bfloat16


def _build_program():
    nc = bacc.Bacc("TRN2", target_bir_lowering=False, debug=False,
                   num_devices=N_CORES)

    # ---- I/O (bf16 inputs pre-tiled on host) --------------------------
    xT = nc.declare_dram_parameter("xT", [D, BS], BF16, isOutput=False)
    wq = nc.declare_dram_parameter("wq", [P, ND * EL], BF16, isOutput=False)
    wk = nc.declare_dram_parameter("wk", [P, ND * EL], BF16, isOutput=False)
    wv = nc.declare_dram_parameter("wv", [P, ND * EL], BF16, isOutput=False)
    wo = nc.declare_dram_parameter("wo", [P, ND * D], BF16, isOutput=False)
    bo = nc.declare_dram_parameter("bo", [1, D], F32, isOutput=False)
    msk = nc.declare_dram_parameter("msk", [P, 2 * P], BF16, isOutput=False)
    out = nc.declare_dram_parameter("out", [2 * QC, D], F32, isOutput=True)

    a2a_in = [nc.dram_tensor(f"a2a_in{b}", [N_CORES, P, QC], BF16)
              for b in range(B)]
    a2a_out = [nc.dram_tensor(f"a2a_out{b}", [N_CORES, P, QC], BF16)
               for b in range(B)]

    with TileContext(nc) as tc, ExitStack() as ctx:
        const = ctx.enter_context(tc.tile_pool(name="const", bufs=1))
        persist = ctx.enter_context(tc.tile_pool(name="persist", bufs=1))
        xtp = ctx.enter_context(tc.tile_pool(name="xt", bufs=8))
        vtp = ctx.enter_context(tc.tile_pool(name="vt", bufs=2))
        probs_pool = ctx.enter_context(tc.tile_pool(name="probs", bufs=7))
        small = ctx.enter_context(tc.tile_pool(name="small", bufs=4))
        osb = ctx.enter_context(tc.tile_pool(name="osb", bufs=4))
        catp = ctx.enter_context(tc.tile_pool(name="cat", bufs=1))
        # PSUM: pp 2 + sc 2*2 + po0 1 + po1 1 = 8 banks
        pp = ctx.enter_context(tc.tile_pool(name="pp", bufs=2, space="PSUM"))
        scp = ctx.enter_context(tc.tile_pool(name="sc", bufs=2, space="PSUM"))
        pop = ctx.enter_context(tc.tile_pool(name="po", bufs=1, space="PSUM"))

        # ---- constants ------------------------------------------------
        identf = const.tile([P, P], F32, tag="identf")
        make_identity(nc, identf)
        ident = const.tile([P, P], BF16, tag="ident")
        nc.vector.tensor_copy(ident[:], identf[:])
        bo_sb = const.tile([1, D], F32, tag="bo_sb")
        bo_bc = const.tile([P, D], F32, tag="bo_bc")
        mask2 = const.tile([P, 2 * P], BF16, tag="mask2")
        w_sb = {}
        for name in ("wq", "wk", "wv"):
            w = const.tile([P, ND * EL], BF16, tag=f"w_{name}",
                           name=f"w_{name}")
            w_sb[name] = w
        wo_sb = const.tile([P, ND * D], BF16, tag="wo_sb")
        warm = const.tile([P, 640], BF16, tag="warm")
        nc.vector.memset(warm[:].bitcast(F32), 0.0)

        def load_consts_early():
            for i, (name, t) in enumerate((("wq", wq), ("wk", wk),
                                           ("wv", wv))):
                dma_engines[i % 3].dma_start(out=w_sb[name][:], in_=t[:])
            nc.scalar.dma_start(out=mask2[:], in_=msk[:])
            nc.sync.dma_start(out=bo_sb[:], in_=bo[:])
            nc.gpsimd.partition_broadcast(bo_bc[:], bo_sb[:])

        def load_consts_late():
            nc.sync.dma_start(out=wo_sb[:], in_=wo[:])

        # persistent activations
        qT = persist.tile([P, BS], BF16, tag="qT")
        kT = persist.tile([P, BS], BF16, tag="kT")
        # vaug[b]: 16 k-tiles of [P, 130]: cols 65h..65h+64 = V_h, 65h+64=1
        vaug = [persist.tile([P, NT * 2 * VW], BF16, tag=f"vaug{b}",
                             name=f"vaug{b}")
                for b in range(B)]
        stage = [persist.tile([P, SBLK], BF16, tag=f"stg{b}{j}",
                              name=f"stg{b}{j}")
                 for b in range(B) for j in range(NJ)]

        mask3 = mask2[:].rearrange("p (h q) -> p h q", h=2)

        # act table warm: tiny exp early so the ~2.7us table load overlaps
        # the initial x DMA
        warm_pr = small.tile([1, 32], F32, tag="wpr")
        nc.scalar.activation(warm_pr[:], warm[0:1, 0:32],
                             mybir.ActivationFunctionType.Exp)

        # ---- emission helpers -----------------------------------------
        warm_ctr = [0]

        def emit_warmup(n):
            for i in range(n):
                warm_ctr[0] += 1
                ps = pp.tile([P, SBLK], F32, tag="pp",
                             name=f"warm{warm_ctr[0]}")
                nc.tensor.matmul(ps[0:P, 0:SBLK], warm[:, 0:128],
                                 warm[:, 128:640], start=True, stop=True)

        dma_engines = [nc.sync, nc.scalar, nc.gpsimd]

        def load_x(b):
            xt = []
            for k in range(ND):
                t = xtp.tile([P, S], BF16, tag="xt")
                dma_engines[k % 3].dma_start(
                    out=t[:], in_=xT[k * P:(k + 1) * P, b * S:(b + 1) * S])
                xt.append(t)
            return xt

        def proj_groups(b, xt, vt, wide=False):
            """Yield once per (name, sb-group): stationary reused across
            2 (interleaved phase) or 4 (solo phase, borrowing sc banks)
            sub-blocks."""
            for sb0, name in ((a, n) for a in (0, 2)
                              for n in ("wq", "wk", "wv")):
                if wide:
                    sct = [scp.tile([P, 2 * SBLK], F32, tag="sc",
                                    name=f"pjw_{b}_{name}_{sb0}_{i}")
                           for i in range(2)]
                    pss = [sct[i][:, j * SBLK:(j + 1) * SBLK]
                           for i in range(2) for j in range(2)]
                    groups = [(sb0, pss)]
                else:
                    t2 = [pp.tile([P, SBLK], F32, tag="pp",
                                  name=f"pj_{b}_{name}_{sb0}_{i}")
                          for i in range(2)]
                    groups = [(sb0, [t[:] for t in t2])]
                for sb0, ps_group in groups:
                    nsb = len(ps_group)
                    for k in range(ND):
                        for i in range(nsb):
                            nc.tensor.matmul(
                                ps_group[i],
                                w_sb[name][:, k * EL:(k + 1) * EL],
                                xt[k][:, (sb0 + i) * SBLK:(sb0 + i + 1) * SBLK],
                                start=(k == 0), stop=(k == ND - 1))
                        if k == 3 and not wide:
                            yield
                    for i in range(nsb):
                        osl = slice(b * S + (sb0 + i) * SBLK,
                                    b * S + (sb0 + i + 1) * SBLK)
                        if name == "wv":
                            nc.vector.tensor_copy(
                                vt[:, (sb0 + i) * SBLK:(sb0 + i + 1) * SBLK],
                                ps_group[i])
                        else:
                            dest = qT if name == "wq" else kT
                            nc.vector.tensor_copy(dest[:, osl], ps_group[i])
                    yield

        def transpose_v(b, vt):
            """Yield once per pair of k-tiles transposed into vaug[b]."""
            va = vaug[b]
            # ones columns first (disjoint from the V columns written below)
            ones = va[:].rearrange("p (t h e) -> p t h e", t=NT, h=2)
            nc.vector.memset(ones[:, :, :, DK:DK + 1], 1.0)
            for t in range(NT):
                ps = pp.tile([P, SBLK], F32, tag="pp", name=f"tr_{b}_{t}")
                psb = ps[:].bitcast(BF16)
                nc.tensor.matmul(psb[:, 0:P], vt[:, t * P:(t + 1) * P],
                                 ident[:], is_transpose=True)
                va3 = va[:, t * 2 * VW:(t + 1) * 2 * VW].rearrange(
                    "p (h e) -> p h e", h=2)
                nc.vector.tensor_copy(
                    va3[:, :, 0:DK],
                    psb[:, 0:P].rearrange("p (h e) -> p h e", h=2))
                if t % 2 == 1:
                    yield

        pend = deque()
        LAG = 4
        po_live = {}

        def emit_attnv(b, j, t, pr3, off, w):
            if t == 0:
                po_live[(b, j)] = {
                    h: pop.tile([P, SBLK], F32, tag=f"po{h}",
                                name=f"po_{b}_{j}_{h}")
                    for h in (0, 1)}
            po = po_live[(b, j)]
            last = (t == 4 * j + 3)
            for h in (0, 1):
                nc.tensor.matmul(
                    po[h][0:VW, off:SBLK],
                    vaug[b][:, (t * 2 + h) * VW:(t * 2 + h) * VW + VW],
                    pr3[:, h, off:SBLK],
                    start=(t == 0), stop=last)
            if last:
                emit_norm(b, j, po)
                del po_live[(b, j)]

        def emit_norm(b, j, po):
            import os
            st = stage[b * NJ + j]
            for h in (0, 1):
                if os.environ.get("KDBG", "") == "raw":
                    nc.vector.tensor_copy(st[h * DK:(h + 1) * DK, :],
                                          po[h][0:DK, :])
                    continue
                if os.environ.get("KDBG", "") == "den":
                    nc.vector.tensor_copy(
                        st[h * DK:h * DK + 1, :], po[h][DK:DK + 1, :])
                    continue
                sm = small.tile([1, SBLK], F32, tag="sm")
                nc.vector.tensor_copy(sm[:], po[h][DK:DK + 1, :])
                rb1 = small.tile([1, SBLK], F32, tag="rb1")
                nc.vector.reciprocal_approx_fast(out=rb1[:], in_=sm[:])
                rbb = small.tile([DK, SBLK], F32, tag="rbb")
                nc.gpsimd.partition_broadcast(rbb[:], rb1[:])
                nc.vector.tensor_mul(st[h * DK:(h + 1) * DK, :],
                                     po[h][0:DK, :], rbb[:])
            for half in (0, 1):
                nc.sync.dma_start(
                    out=a2a_in[b][2 * j + half],
                    in_=st[:, half * QC:(half + 1) * QC])

        def sweep_units(b):
            """Yield once per (j, t) unit: score pair + exp (+mask); attnV
            lags via pend."""
            for j in range(NJ):
                for t in range(4 * j + 4):
                    r = t - 4 * j
                    off = max(0, r) * P
                    w = SBLK - off
                    sc = scp.tile([P, 2 * SBLK], F32, tag="sc",
                                  name=f"sc_{b}_{j}_{t}")
                    sc3 = sc[:].rearrange("p (h q) -> p h q", h=2)
                    q0 = b * S + j * SBLK + off
                    k0 = b * S + t * P
                    for h in (0, 1):
                        nc.tensor.matmul(
                            sc3[:, h, off:SBLK],
                            kT[h * DK:(h + 1) * DK, k0:k0 + P],
                            qT[h * DK:(h + 1) * DK, q0:q0 + w],
                            start=True, stop=True)
                    pr = probs_pool.tile([P, 2 * SBLK], BF16, tag="probs")
                    pr3 = pr[:].rearrange("p (h q) -> p h q", h=2)
                    nc.scalar.activation(pr3[:, :, off:SBLK],
                                         sc3[:, :, off:SBLK],
                                         mybir.ActivationFunctionType.Exp)
                    if r >= 0:
                        nc.vector.tensor_mul(pr3[:, :, off:off + P],
                                             pr3[:, :, off:off + P],
                                             mask3)
                    pend.append((b, j, t, pr3, off, w))
                    # previous q-block's attnVs are past their exps: drain
                    # them now so the new block's first attnV (which waits
                    # on the normalize chain freeing the po banks) sits
                    # deep enough in the FIFO to hide that chain
                    while pend and pend[0][1] != j:
                        emit_attnv(*pend.popleft())
                    while len(pend) > LAG:
                        emit_attnv(*pend.popleft())
                    yield

        def drain():
            while pend:
                emit_attnv(*pend.popleft())

        def load_cat(b, engines):
            cat = []
            for s in range(N_CORES):
                t = catp.tile([P, QC], BF16, tag=f"cat{b}{s}",
                              name=f"cat{b}{s}")
                engines[s % len(engines)].dma_start(out=t[:],
                                                    in_=a2a_out[b][s])
                cat.append(t)
            return cat

        def outproj(b, cat):
            """Yield per (st, eb) block of the output projection for batch
            b's 2x128 rows."""
            for st in (0, 1):
                pss = [pp.tile([P, SBLK], F32, tag="pp",
                               name=f"op_{b}_{st}_{eb}") for eb in (0, 1)]
                for s in range(N_CORES):
                    for eb in (0, 1):
                        nc.tensor.matmul(
                            pss[eb][:], cat[s][:, st * P:(st + 1) * P],
                            wo_sb[:, s * D + eb * SBLK:s * D + (eb + 1) * SBLK],
                            start=(s == 0), stop=(s == N_CORES - 1))
                for eb in (0, 1):
                    ot = osb.tile([P, SBLK], F32, tag="osb")
                    nc.vector.tensor_add(ot[:], pss[eb][:],
                                         bo_bc[:, eb * SBLK:(eb + 1) * SBLK])
                    nc.sync.dma_start(
                        out=out[b * QC + st * P:b * QC + (st + 1) * P,
                                eb * SBLK:(eb + 1) * SBLK],
                        in_=ot[:])
                    yield

        # ---- master schedule ------------------------------------------
        xt0 = load_x(0)
        load_consts_early()
        emit_warmup(8)
        vt0 = vtp.tile([P, S], BF16, tag="vt", name="vt0")
        pj0 = proj_groups(0, xt0, vt0)
        tv0 = transpose_v(0, vt0)
        sw0 = sweep_units(0)
        # sb0/sb1 projections + their transposes first, so sweep-b0's first
        # two q-blocks can run inside the projection phase
        for _ in range(6):
            next(pj0, None)
        for _ in range(4):
            next(tv0, None)

        xt1 = load_x(1)
        load_consts_late()
        vt1 = vtp.tile([P, S], BF16, tag="vt", name="vt1")
        import os
        KSEQ = os.environ.get("KSEQ", "0")
        SEQ = KSEQ in ("1", "3")      # sequential proj-b1/tv1
        SEQ_OP = KSEQ in ("1", "2")   # sequential outproj-b0
        pj1 = proj_groups(1, xt1, vt1)
        tv1 = transpose_v(1, vt1)
        # units 0-11 (q-blocks 0-1) interleave with the sb2/sb3 projections
        items0 = [pj0] * 6 + [tv0] * 4
        for i in range(12):
            next(sw0, None)
            if i < len(items0):
                next(items0[i], None)
        for _ in pj0:
            pass
        for _ in tv0:
            pass
        if SEQ:
            for _ in pj1:
                pass
            for _ in tv1:
                pass
            for _ in sw0:
                pass
        else:
            items1 = [pj1] * 6 + [tv1] * 4 + [pj1] * 6 + [tv1] * 4
            for i in range(28):
                next(sw0, None)
                if i < len(items1):
                    next(items1[i], None)
            for _ in sw0:
                pass
            for _ in pj1:
                pass
            for _ in tv1:
                pass
        drain()
        nc.gpsimd.collective_compute(
            "AllToAll", mybir.AluOpType.bypass,
            replica_groups=[list(range(N_CORES))],
            ins=[a2a_in[0][:]], outs=[a2a_out[0][:]])
        cat0 = load_cat(0, [nc.sync])

        # sweep b1 interleaved with the deferred sb23 projections and
        # V-transposes (fills the Act-bound phase's PE bubbles); then a2a#B
        # launches while outproj b0 (whose collective landed mid-sweep)
        # keeps the PE busy
        for _ in sweep_units(1):
            pass
        drain()
        nc.gpsimd.collective_compute(
            "AllToAll", mybir.AluOpType.bypass,
            replica_groups=[list(range(N_CORES))],
            ins=[a2a_in[1][:]], outs=[a2a_out[1][:]])
        cat1 = load_cat(1, [nc.sync, nc.scalar])
        tc.no_sync_barrier()
        for _ in outproj(0, cat0):
            pass
        for _ in outproj(1, cat1):
            pass

        import os
        dbg = os.environ.get("KDBG", "")
        if dbg in ("q", "k", "v", "st", "raw", "den"):
            dbt = osb.tile([P, D], F32, tag="dbg", name="dbt")
            if dbg == "q":
                srcs = [qT[:, 0:D], qT[:, S:S + D]]
            elif dbg == "k":
                srcs = [kT[:, 0:D], kT[:, S:S + D]]
            elif dbg == "v":
                srcs = [vaug[0][:, 0:D], vaug[1][:, 0:D]]
            elif dbg in ("st", "raw", "den"):
                srcs = [stage[0][:], stage[3][:], stage[4][:], stage[7][:]]
            for r, sap in enumerate(srcs):
                w = sap.free_size()
                nc.vector.tensor_copy(dbt[:, 0:w], sap)
                nc.sync.dma_start(out=out[r * P:(r + 1) * P, 0:w],
                                  in_=dbt[:, 0:w])

    nc.compile()
    return nc


def _sbuf_tiled(wT):
    # [D, E] -> [P, ND*E]: row p holds d-chunk k at columns [k*E, (k+1)*E)
    dd, e = wT.shape
    return np.ascontiguousarray(
        wT.reshape(dd // P, P, e).transpose(1, 0, 2).reshape(P, -1))


def _bf16(a):
    import ml_dtypes
    return np.ascontiguousarray(a).astype(ml_dtypes.bfloat16)


def _prepare_inputs(x, Wq, Wk, Wv, Wo, bo):
    x = np.asarray(x, np.float32)
    xT = np.ascontiguousarray(
        np.concatenate([x[b].T for b in range(B)], axis=1))
    xT = _bf16(xT)
    woT = _bf16(_sbuf_tiled(np.ascontiguousarray(np.asarray(Wo,
                                                            np.float32).T)))
    bo2 = np.asarray(bo, np.float32).reshape(1, D)
    scale = np.float32(1.0 / np.sqrt(DK))
    # causal triangle in [k, q] layout: valid iff q >= k; two copies side
    # by side so one DVE multiply masks both heads
    tri = np.triu(np.ones((P, P), np.float32))
    msk = _bf16(np.concatenate([tri, tri], axis=1))
    in_maps = []
    for c in range(N_CORES):
        cols = slice(c * EL, (c + 1) * EL)
        in_maps.append({
            "xT": xT,
            "wq": _bf16(_sbuf_tiled(np.asarray(Wq, np.float32).T[:, cols]
                                    * scale)),
            "wk": _bf16(_sbuf_tiled(np.asarray(Wk, np.float32).T[:, cols])),
            "wv": _bf16(_sbuf_tiled(np.asarray(Wv, np.float32).T[:, cols])),
            "wo": woT,
            "bo": bo2,
            "msk": msk,
        })
    return in_maps


def _run(inputs, trace=False):
    nc = _build_program()
    in_maps = _prepare_inputs(inputs["x"], inputs["Wq"], inputs["Wk"],
                              inputs["Wv"], inputs["Wo"], inputs["bo"])
    res = run_bass_kernel_spmd(nc, in_maps, list(range(N_CORES)),
                               trace=trace)
    full = np.empty((B, S, D), np.float32)
    for c in range(N_CORES):
        r = res.results[c]["out"]
        for b in range(B):
            full[b, c * QC:(c + 1) * QC, :] = r[b * QC:(b + 1) * QC]
    return full, res


def kernel(**inputs) -> np.ndarray:
    out, _ = _run(inputs, trace=False)
    return out
